# revision 3
# baseline (speedup 1.0000x reference)
"""Multi-head attention Trainium2 kernel (B=4, N=2048, D=1024, H=16).

Sharding: 8 cores = 4 batches x 2 head-groups (8 heads each), zero
collectives.  The kernel is paced by the ACT engine: 256 exp ACTIVATEs
([128,1024] each, ~1.2us steady issue rate) = ~311us floor; every other
engine hides behind that stream.

Per core, per unit (chunk c of 512 queries x head-pair p):
  - S: two row-tiled matmuls (head A PE rows 0:64 / head B rows 64:128)
    into one [128,1024] PSUM tile per key block m (bufs=2 pipeline).
  - exp on ACT, fp16 out.
  - PV: col-tiled matmul pair (A -> PSUM partitions 0:64 / PE cols 0:63,
    B -> 64:128), concurrent streams (measured 222ns/pair), lag 9 blocks
    behind S.
  - denominator: exp tiles accumulated into acc[128,1024] (DVE + 2
    GPSIMD adds per unit); per pair-of-units one PSUM collector gets 4
    col-tiled ones-matmul dens at partitions {0,32,64,96}; ONE [97,512]
    reciprocal serves all 4 rows (reciprocal is free-dim-bound); K=1
    ones-matmuls broadcast the recip rows, one DVE multiply normalizes
    straight out of PV PSUM.
  - x is staged as two [128,4,2048] tiles (4KB DMA runs, single trigger
    each); late weights are gated behind the first k-proj so x gets the
    HBM bandwidth first.  All projections (k/v/q) and the out-projection
    are emitted just-in-time as PE filler between S steps; the unit order
    interleaves chunk-0/chunk-1 pairs to spread the projection bulge.

Host sums the two head-group partials per batch and adds bias.
"""
from contextlib import ExitStack

import numpy as np

import concourse.mybir as mybir
import concourse.tile as tile
from concourse import bacc
from concourse.bass_utils import run_bass_kernel_spmd

F32 = mybir.dt.float32
F16 = mybir.dt.float16

P = 128
N = 2048         # sequence length
DI = 1024        # model dim
NH = 8           # heads per core
HD = 64          # head dim
NPAIR = 4        # head pairs per core
KT = 8           # contraction tiles for projections
CH = 512         # query chunk width
NCHUNK = 4       # chunks per sequence
MT = 16          # key tiles (m) per sequence
ET = 8           # output-feature blocks
SCALE = HD ** -0.5

ACC_GP = (6, 12)     # m-steps whose denominator add runs on GPSIMD
PV_LAG = 9

# position -> (chunk, pair): interleaved so chunk-0's projection load
# spreads over 8 units while chunk results still finish early enough
# for the out-projection.
UNITS = [(0, 0), (0, 1), (1, 0), (1, 1), (0, 2), (0, 3), (1, 2), (1, 3),
         (2, 0), (2, 1), (2, 2), (2, 3), (3, 0), (3, 1), (3, 2), (3, 3)]

_NC_CACHE = None


def _build():
    nc = bacc.Bacc("TRN2", target_bir_lowering=False, debug=False)

    xT = nc.dram_tensor("xT", [DI, N], F16, kind="ExternalInput").ap()
    wqkA = nc.dram_tensor("wqkA", [8, P, KT, P], F16, kind="ExternalInput").ap()
    wvA = nc.dram_tensor("wvA", [P, KT, 512], F16, kind="ExternalInput").ap()
    woT = nc.dram_tensor("woT", [512, DI], F16, kind="ExternalInput").ap()
    onesd = nc.dram_tensor("ones", [P, HD], F16, kind="ExternalInput").ap()
    outT = nc.dram_tensor("outT", [DI, N], F16, kind="ExternalOutput").ap()

    xT_k = xT.rearrange("(k p) n -> p k n", p=P)        # [128, 8, 2048]
    wqk_k = wqkA.rearrange("f p k n -> p f k n")        # [128, 8, KT, 128]
    woT_r = woT.rearrange("(k p) e -> p k e", p=P)      # [128, 4, 1024]
    outT_r = outT.rearrange("(e p) n -> e p n", p=P)    # [8, 128, 2048]

    with tile.TileContext(nc) as tc, ExitStack() as persist:
        # ---------------- persistent SBUF ----------------
        qk_pool = persist.enter_context(tc.tile_pool(name="qkp", bufs=8))
        v_pool = persist.enter_context(tc.tile_pool(name="vp", bufs=1))
        misc = persist.enter_context(tc.tile_pool(name="misc", bufs=2))
        wkq_pool = persist.enter_context(tc.tile_pool(name="wkq", bufs=1))
        xt_pool = persist.enter_context(tc.tile_pool(name="xt", bufs=2))
        wq_pool = persist.enter_context(tc.tile_pool(name="wq", bufs=1))
        wk_pool = persist.enter_context(tc.tile_pool(name="wk", bufs=1))
        wv_pool = persist.enter_context(tc.tile_pool(name="wv", bufs=1))
        wo_pool = persist.enter_context(tc.tile_pool(name="wo", bufs=1))

        exp_pool = persist.enter_context(tc.tile_pool(name="expp", bufs=17))
        acc_pool = persist.enter_context(tc.tile_pool(name="accp", bufs=3))
        rcp_pool = persist.enter_context(tc.tile_pool(name="rcpp", bufs=2))
        rb_pool = persist.enter_context(tc.tile_pool(name="rbp", bufs=3))
        ot_pool = persist.enter_context(tc.tile_pool(name="ot", bufs=10))
        st_pool = persist.enter_context(tc.tile_pool(name="st", bufs=4))

        sps_pool = persist.enter_context(
            tc.tile_pool(name="sps", bufs=2, space="PSUM"))
        oaug_pool = persist.enter_context(
            tc.tile_pool(name="oaug", bufs=2, space="PSUM"))
        aux_pool = persist.enter_context(
            tc.tile_pool(name="aux", bufs=2, space="PSUM"))

        ones_r = misc.tile([P, HD], F16)
        nc.sync.dma_start(ones_r[:], onesd[:])
        # dummy exp: pull ACT_TABLE_LOAD into the DMA window
        scr = misc.tile([P, HD], F16)
        nc.scalar.activation(scr[:], ones_r[:],
                             mybir.ActivationFunctionType.Exp, scale=0.01)

        qkT = [qk_pool.tile([P, N], F16, name=f"qkT{t}", tag="qkT")
               for t in range(8)]
        vT = v_pool.tile([P, MT, NH, HD], F16)

        # x as two big tiles [128, 4, 2048] (4KB contiguous runs -> full
        # HBM rate, single DMA trigger each; triggers cost ~650ns on the
        # sync queue so batching matters).  Late weights are batched the
        # same way and gated behind the first k-proj output so they don't
        # steal bandwidth from xT.
        xth = []
        for h in range(2):
            t = xt_pool.tile([P, 4, N], F16, name=f"xth{h}", tag="xt")
            if h == 1:
                # gate: second half queues only after the first lands, so
                # the first k-proj terms start a DMA-half earlier.
                nc.vector.tensor_copy(t[0:1, 0:1, 0:1], xth[0][0:1, 0:1, 0:1])
            nc.sync.dma_start(t[:], xT_k[:, 4 * h:4 * h + 4, :])
            xth.append(t)

        def xts(k):
            return xth[k // 4][:, k % 4]

        wkq0 = wkq_pool.tile([P, 2, KT, P], F16)      # [k-pair0 | q-pair0]
        nc.sync.dma_start(wkq0[:, 0], wqk_k[:, 4])
        nc.sync.dma_start(wkq0[:, 1], wqk_k[:, 0])
        wv = wv_pool.tile([P, KT, 512], F16)
        wk123 = wk_pool.tile([P, 3, KT, P], F16)
        wq123 = wq_pool.tile([P, 3, KT, P], F16)
        woA = wo_pool.tile([P, 4, DI], F16)

        def wk_(p):
            return wkq0[:, 0] if p == 0 else wk123[:, p - 1]

        def wq_(f):
            return wkq0[:, 1] if f == 0 else wq123[:, f - 1]

        def dma_late_weights():
            """Gated DMAs: a marker write depending on the head k-proj
            output makes each transfer queue behind the x loads."""
            def gate(dst2d):
                nc.vector.tensor_copy(dst2d, qkT[4][0:1, 0:1])

            gate(wv[0:1, 0:1, 0:1])
            nc.sync.dma_start(wv[:], wvA[:])
            gate(wk123[0:1, 0:1, 0:1, 0:1])
            nc.sync.dma_start(wk123[:], wqk_k[:, 5:8])
            gate(wq123[0:1, 0:1, 0:1, 0:1])
            nc.sync.dma_start(wq123[:], wqk_k[:, 1:4])
            gate(woA[0:1, 0:1, 0:1])
            nc.sync.dma_start(woA[:], woT_r[:])

        state = {}     # position -> dict
        dcol_cur = {}  # group (pos//2) -> collector tile

        # ---------------- emission helpers ----------------
        def emit_kproj(p, cc):
            csl = slice(cc * CH, (cc + 1) * CH)
            ps = aux_pool.tile([P, CH], F32, tag="aux", name=f"kp_{p}_{cc}")
            for k in range(KT):
                nc.tensor.matmul(ps[:], wk_(p)[:, k, :], xts(k)[:, csl],
                                 start=(k == 0), stop=(k == KT - 1))
            nc.vector.tensor_copy(qkT[4 + p][:, csl], ps[:])

        def emit_qproj(c, f):
            csl = slice(c * CH, (c + 1) * CH)
            ps = aux_pool.tile([P, CH], F32, tag="aux", name=f"qp_{c}_{f}")
            for k in range(KT):
                nc.tensor.matmul(ps[:], wq_(f)[:, k, :], xts(k)[:, csl],
                                 start=(k == 0), stop=(k == KT - 1))
            nc.vector.tensor_copy(qkT[f][:, csl], ps[:])

        def emit_vproj(r, half):
            """v projection row-block r, half 0 = pairs 0-1, 1 = pairs 2-3."""
            fsl = slice(half * 256, (half + 1) * 256)
            ps = aux_pool.tile([P, 256], F32, tag="aux", name=f"vp_{r}_{half}")
            for k in range(KT):
                nc.tensor.matmul(ps[:], xts(k)[:, r * P:(r + 1) * P],
                                 wv[:, k, fsl],
                                 start=(k == 0), stop=(k == KT - 1))
            nc.vector.tensor_copy(
                vT[:, r, 4 * half:4 * half + 4, :],
                ps.rearrange("p (h d) -> p h d", d=HD))

        def begin_unit(u):
            c, p = UNITS[u]
            state[u] = {
                "accU": acc_pool.tile([P, 2 * CH], F16, tag="accp",
                                      name=f"acc_{c}_{p}"),
                "oaug": oaug_pool.tile([P, CH], F32, tag="oaug",
                                       name=f"oaug_{c}_{p}"),
                "expPs": [],
            }

        def emit_S(u, m):
            c, p = UNITS[u]
            csl = slice(c * CH, (c + 1) * CH)
            msl = slice(m * P, (m + 1) * P)
            st_ = state[u]
            sps = sps_pool.tile([P, 2 * CH], F32, tag="sps",
                                name=f"sps_{c}_{p}_{m}")
            kTl = qkT[4 + p]
            nc.tensor.matmul(sps[:, 0:CH], kTl[0:HD, msl],
                             qkT[p][0:HD, csl], start=True, stop=True)
            nc.tensor.matmul(sps[:, CH:2 * CH], kTl[HD:P, msl],
                             qkT[p][HD:P, csl], start=True, stop=True)
            expP = exp_pool.tile([P, 2 * CH], F16, tag="expp",
                                 name=f"expP_{c}_{p}_{m}")
            nc.scalar.activation(expP[:], sps[:],
                                 mybir.ActivationFunctionType.Exp,
                                 scale=SCALE)
            st_["expPs"].append(expP)
            accU = st_["accU"]
            with nc.allow_low_precision(reason="softmax denom accum"):
                if m == 0:
                    nc.vector.tensor_copy(accU[:], expP[:])
                elif m in ACC_GP:
                    nc.gpsimd.tensor_tensor(accU[:], accU[:], expP[:],
                                            mybir.AluOpType.add)
                else:
                    nc.vector.tensor_tensor(accU[:], accU[:], expP[:],
                                            mybir.AluOpType.add)

        def emit_PV(u, m):
            c, p = UNITS[u]
            st_ = state[u]
            expP = st_["expPs"][m]
            oaug = st_["oaug"]
            nc.tensor.matmul(oaug[0:HD, :], vT[:, m, 2 * p, :],
                             expP[:, 0:CH],
                             start=(m == 0), stop=(m == MT - 1))
            nc.tensor.matmul(oaug[HD:P, :], vT[:, m, 2 * p + 1, :],
                             expP[:, CH:2 * CH],
                             start=(m == 0), stop=(m == MT - 1))

        def emit_den(u):
            """den ones-matmuls for unit u into the 2-unit collector.
            Even u -> partitions {0,32}; odd u -> {64,96}."""
            grp = u // 2
            if u % 2 == 0:
                dcol_cur[grp] = aux_pool.tile([P, CH], F32, tag="aux",
                                              name=f"dcol_{grp}")
            dcol = dcol_cur[grp]
            base = 64 * (u % 2)
            accU = state[u]["accU"]
            nc.tensor.matmul(dcol[base:base + 1, :], ones_r[:, 0:1],
                             accU[:, 0:CH], start=True, stop=True,
                             tile_position=(0, base))
            nc.tensor.matmul(dcol[base + 32:base + 33, :], ones_r[:, 0:1],
                             accU[:, CH:2 * CH], start=True, stop=True,
                             tile_position=(0, base + 32))

        def emit_recip(grp, quarter=None):
            """reciprocal of the 2-unit den collector; split into 128-col
            quarters so the DVE queue never eats one 4us bubble."""
            if quarter is None or quarter == 0:
                rcp = rcp_pool.tile([97, CH], F16, tag="rcpp",
                                    name=f"rcp_{grp}")
                state[2 * grp]["rcp"] = rcp
                if 2 * grp + 1 in state:
                    state[2 * grp + 1]["rcp"] = rcp
            else:
                rcp = state[2 * grp]["rcp"]
            qs = range(4) if quarter is None else [quarter]
            with nc.allow_low_precision(reason="softmax denom"):
                for qq in qs:
                    fsl = slice(qq * P, (qq + 1) * P)
                    nc.vector.reciprocal(rcp[0:97, fsl],
                                         dcol_cur[grp][0:97, fsl])

        def emit_norm(u):
            c, p = UNITS[u]
            st_ = state[u]
            rcp = st_["rcp"]
            base = 64 * (u % 2)
            bc = aux_pool.tile([P, CH], F32, tag="aux", name=f"bc_{c}_{p}")
            nc.tensor.matmul(bc[0:HD, :], ones_r[base:base + 1, 0:HD],
                             rcp[base:base + 1, :], start=True, stop=True,
                             tile_position=(base, 0))
            nc.tensor.matmul(bc[HD:P, :], ones_r[base + 32:base + 33, 0:HD],
                             rcp[base + 32:base + 33, :],
                             start=True, stop=True,
                             tile_position=(base + 32, HD))
            rb = rb_pool.tile([P, CH], F16, tag="rbp", name=f"rb_{c}_{p}")
            nc.vector.tensor_copy(rb[:], bc[:])
            ot_p = ot_pool.tile([P, CH], F16, tag="ot", name=f"ot_{c}_{p}")
            nc.vector.tensor_tensor(ot_p[:], st_["oaug"][:], rb[:],
                                    mybir.AluOpType.mult)
            st_["ot"] = ot_p
            st_["expPs"] = None

        pos_of = {cp: u for u, cp in enumerate(UNITS)}

        def emit_outproj_e(c, e):
            csl = slice(c * CH, (c + 1) * CH)
            pso = aux_pool.tile([P, CH], F32, tag="aux", name=f"pso_{c}_{e}")
            for p in range(NPAIR):
                nc.tensor.matmul(pso[:], woA[:, p, e * P:(e + 1) * P],
                                 state[pos_of[(c, p)]]["ot"][:],
                                 start=(p == 0), stop=(p == NPAIR - 1))
            stt = st_pool.tile([P, CH], F16, tag="st", name=f"st_{c}_{e}")
            nc.vector.tensor_copy(stt[:], pso[:])
            nc.sync.dma_start(outT_r[e][:, csl], stt[:])

        # ---------------- filler schedule ----------------
        fillers = [[[] for _ in range(MT)] for _ in range(len(UNITS))]

        def add_filler(u, m, fn):
            fillers[u][m].append(fn)

        # k projections, staggered by chunk-column just-in-time.
        for cc in range(1, 4):
            add_filler(0, (1, 5, 9)[cc - 1], lambda cc=cc: emit_kproj(0, cc))
        kplace = {1: (0, 1), 2: (3, 4), 3: (4, 5)}   # p -> (cc0 unit, rest unit)
        for p in range(1, 4):
            u0, u1 = kplace[p]
            add_filler(u0, 13, lambda p=p: emit_kproj(p, 0))
            for cc in range(1, 4):
                add_filler(u1, (1, 6, 10)[cc - 1],
                           lambda p=p, cc=cc: emit_kproj(p, cc))

        # v rows: half 0 over units 0-1, half 1 over units 2-3 (unit 2's
        # rows start at step 7 so the aux rotation never evicts the den
        # collector before its reciprocal).
        for r in range(MT):
            if r < 12:
                add_filler(0, 4 + r, lambda r=r: emit_vproj(r, 0))
            else:
                add_filler(1, r - 12, lambda r=r: emit_vproj(r, 0))
        for r in range(MT):
            if r < 8:
                add_filler(2, (7, 8, 9, 11, 12, 13, 14, 15)[r],
                           lambda r=r: emit_vproj(r, 1))
            else:
                add_filler(3, (1, 3, 5, 7, 9, 11, 13, 15)[r - 8],
                           lambda r=r: emit_vproj(r, 1))

        # q projections: block for the consumer at position u+1 emitted at
        # position u (chunk 0 f0 is in the head).
        add_filler(0, 3, lambda: emit_qproj(0, 1))
        for u in range(1, len(UNITS) - 1):
            c_n, p_n = UNITS[u + 1]
            if (c_n, p_n) == (0, 0) or (c_n == 0 and p_n == 1):
                continue
            add_filler(u, 4, lambda c=c_n, f=p_n: emit_qproj(c, f))

        # den/recip/norm per pair of units at even position w:
        #   den(w-2)@s1, den(w-1)@s2, recip@s3 (runs ~4us on DVE),
        #   norm(w-2)@s7, norm(w-1)@s11.
        for w in range(2, len(UNITS), 2):
            add_filler(w, 1, lambda u=w - 2: emit_den(u))
            add_filler(w, 2, lambda u=w - 1: emit_den(u))
            for qq in range(4):
                add_filler(w, 3 + qq,
                           lambda g=(w - 2) // 2, qq=qq: emit_recip(g, qq))
            add_filler(w, 7, lambda u=w - 2: emit_norm(u))
            add_filler(w, 11, lambda u=w - 1: emit_norm(u))
        add_filler(15, 1, lambda: emit_den(14))

        # outproj: one unit of slack after the last norm of each chunk so
        # the pso matmuls never head-of-line block on a sliding DVE.
        oplace = {0: (9, 10), 1: (11, 12), 2: (14, 15)}
        for c, (ua, ub) in oplace.items():
            for e in range(ET):
                u = ua if e < 4 else ub
                add_filler(u, (5, 8, 12, 15)[e % 4],
                           lambda c=c, e=e: emit_outproj_e(c, e))

        # ---------------- head ----------------
        # kproj(0,cc0) and qproj(0,f0) interleaved per k-term: both finish
        # right after the last x half-tile lands.
        kp0 = aux_pool.tile([P, CH], F32, tag="aux", name="kp_0_0")
        qp0 = aux_pool.tile([P, CH], F32, tag="aux", name="qp_0_0")
        for k in range(KT):
            nc.tensor.matmul(kp0[:], wk_(0)[:, k, :], xts(k)[:, 0:CH],
                             start=(k == 0), stop=(k == KT - 1))
            nc.tensor.matmul(qp0[:], wq_(0)[:, k, :], xts(k)[:, 0:CH],
                             start=(k == 0), stop=(k == KT - 1))
        nc.vector.tensor_copy(qkT[4][:, 0:CH], kp0[:])
        nc.vector.tensor_copy(qkT[0][:, 0:CH], qp0[:])
        dma_late_weights()

        # ---------------- main unit loop ----------------
        pv_backlog = []
        for u in range(len(UNITS)):
            begin_unit(u)
            for m in range(MT):
                emit_S(u, m)
                if pv_backlog:
                    uu, mm = pv_backlog.pop(0)
                    emit_PV(uu, mm)
                if m >= PV_LAG:
                    emit_PV(u, m - PV_LAG)
                for fn in fillers[u][m]:
                    fn()
            for m in range(MT - PV_LAG, MT):
                pv_backlog.append((u, m))

        # ---------------- tail ----------------
        for (uu, mm) in pv_backlog:
            emit_PV(uu, mm)
        emit_den(15)
        emit_recip(7)
        emit_norm(14)
        emit_norm(15)
        for e in range(ET):
            emit_outproj_e(3, e)

    nc.compile()
    return nc


def _get_nc():
    global _NC_CACHE
    if _NC_CACHE is None:
        _NC_CACHE = _build()
    return _NC_CACHE


def _make_in_maps(x, w_qkv, w_out):
    ones = np.ones((P, HD), dtype=np.float16)
    per_g = []
    for g in range(2):
        qk_g = np.concatenate([w_qkv[g * 512:(g + 1) * 512],
                               w_qkv[DI + g * 512:DI + (g + 1) * 512]], axis=0)
        wqkT = np.ascontiguousarray(qk_g.T)               # [1024 d, 1024 f]
        wqkA = np.ascontiguousarray(
            wqkT.reshape(KT, P, 8, P).transpose(2, 1, 0, 3).astype(np.float16))
        v_g = w_qkv[2 * DI + g * 512:2 * DI + (g + 1) * 512]
        wvT = np.ascontiguousarray(v_g.T)                 # [1024 d, 512 f]
        wvA = np.ascontiguousarray(
            wvT.reshape(KT, P, 512).transpose(1, 0, 2).astype(np.float16))
        woTg = np.ascontiguousarray(
            w_out[:, g * 512:(g + 1) * 512].T.astype(np.float16))
        per_g.append((wqkA, wvA, woTg))

    in_maps = []
    for core in range(8):
        b, g = core // 2, core % 2
        wqkA, wvA, woTg = per_g[g]
        in_maps.append({
            "xT": np.ascontiguousarray(x[b].T.astype(np.float16)),
            "wqkA": wqkA,
            "wvA": wvA,
            "woT": woTg,
            "ones": ones,
        })
    return in_maps


def kernel(x, w_qkv, w_out, b_out):
    x = np.asarray(x, dtype=np.float32)
    w_qkv = np.asarray(w_qkv, dtype=np.float32)
    w_out = np.asarray(w_out, dtype=np.float32)
    b_out = np.asarray(b_out, dtype=np.float32)
    B = x.shape[0]

    in_maps = _make_in_maps(x, w_qkv, w_out)
    nc = _get_nc()
    res = run_bass_kernel_spmd(nc, in_maps, core_ids=list(range(8)))
    parts = [r["outT"] for r in res.results]
    out = np.empty((B, N, DI), dtype=np.float32)
    for b in range(B):
        out[b] = (parts[2 * b].astype(np.float32)
                  + parts[2 * b + 1].astype(np.float32)).T + b_out
    return out
